# revision 3
# baseline (speedup 1.0000x reference)
"""Trainium2 Bass kernel for nn_Attention_34119220199768 (sparse attention).

Data-parallel over batch B=16 across 8 NeuronCores (2 batches/core).

Per batch b the reference computes
    qs  = query @ Wq + bq            [S,W,Din]
    k_  = data @ Wk + bk             [S,Din]
    vs  = data @ Wv + bv             [S,Dout]
    att = softmax_k(mask(qs . k_ / sqrt(Din)))   [q,k,w]
    zs  = (att @ vs) @ Wz + bz       [S,W,Dout]

Device-side restructuring (everything fp32; matmuls in fp32r = full-rate):
    kT   = Wk^T @ dataT  (+bk)       [Din,S]    (dataT host-pretransposed)
    vsT  = Wv^T @ dataT  (+bv)       [Dout,S]
    WqK  = Wq @ kT                   [DQ,S]     -- scores = query@(Wq@kT)
    bqk  = bq @ kT                   [1,S]
    VZ   = vsT^T-contract Wz         [S,Dout]   -- zs = P@(vs@Wz)
    per 128-row tile of (q,w):
      s   = queryT^T @ WqK (+bqk via K=1 matmul)         [128,S]
      t   = (s + KMASK)*cm*wm  (masked, shifted)         [128,S]
      P   = exp((t - max_k t)/sqrt(Din))  (unnormalized) [128,S]
      PT  = P^T  (TensorE transpose)                     [S,128]
      zs_un = PT^T-contract VZ                            [128,Dout]
Host finishes: row sums, att = P/sum, zs = zs_un/sum + bz.
Fully-masked rows come out uniform (P=1 each, sum=S) exactly like the
reference's softmax over all -1e9.
"""

import numpy as np

import concourse.bass as bass
import concourse.mybir as mybir
import concourse.tile as tile
from concourse import bacc
from concourse import bass_utils
from concourse.masks import make_identity

F32 = mybir.dt.float32
F32R = mybir.dt.float32r

N_CORES = 8
B, S, W, DQ, DIN, DOUT = 16, 512, 8, 256, 512, 512
BPC = B // N_CORES          # batches per core
QW = S * W                  # 4096 flattened (q, w) rows
K_MASK = 3400.0             # shift: exp((0 - (K+max_s))/sqrt(DIN)) underflows to 0
INV = 1.0 / float(np.sqrt(np.float32(DIN)))

N_SUB = QW // 128           # 32 row-subtiles per batch
SUB_PER_CH = 4              # subtiles per query chunk
N_CH = N_SUB // SUB_PER_CH  # 8 chunks per batch


def build():
    nc = bacc.Bacc("TRN2", target_bir_lowering=False, debug=False,
                   num_devices=N_CORES)

    def din(name, shape, dt=F32R):
        return nc.dram_tensor(name, shape, dt, kind="ExternalInput").ap()

    qT = din("queryT", [BPC, DQ, QW])
    dT = din("dataT", [BPC, DIN, S])
    wqt = din("WqT", [DIN, DQ])
    wk = din("Wk", [DIN, DIN])
    wv = din("Wv", [DIN, DOUT])
    wz = din("Wz", [DOUT, DOUT])
    bq = din("bq", [DIN])
    bk = din("bk", [DIN], F32)
    bv = din("bv", [DOUT], F32)
    cm = din("cm", [BPC, QW], F32)
    wm = din("wm", [BPC, S], F32)
    att_o = nc.dram_tensor("att", [BPC, QW, S], F32, kind="ExternalOutput").ap()
    zs_o = nc.dram_tensor("zs", [BPC, QW, DOUT], F32, kind="ExternalOutput").ap()

    from contextlib import ExitStack
    with tile.TileContext(nc) as tc, ExitStack() as ctx:
        sing = ctx.enter_context(tc.tile_pool(name="sing", bufs=1))
        batch_p = ctx.enter_context(tc.tile_pool(name="batch", bufs=2))
        chunk_p = ctx.enter_context(tc.tile_pool(name="chunk", bufs=3))
        sub_p = ctx.enter_context(tc.tile_pool(name="sub", bufs=3))
        stat_p = ctx.enter_context(tc.tile_pool(name="stat", bufs=6))
        ps_set = ctx.enter_context(tc.tile_pool(name="ps_set", bufs=2, space="PSUM"))
        ps_sc = ctx.enter_context(tc.tile_pool(name="ps_sc", bufs=2, space="PSUM"))
        ps_pt = ctx.enter_context(tc.tile_pool(name="ps_pt", bufs=2, space="PSUM"))
        ps_zs = ctx.enter_context(tc.tile_pool(name="ps_zs", bufs=2, space="PSUM"))

        # ---- one-time weights / constants ----
        wqt_s = sing.tile([128, 4, DQ], F32R)
        nc.sync.dma_start(out=wqt_s, in_=wqt.rearrange("(t p) d -> p t d", p=128))
        wk_s = sing.tile([128, 4, DIN], F32R)
        nc.sync.dma_start(out=wk_s, in_=wk.rearrange("(t p) i -> p t i", p=128))
        wv_s = sing.tile([128, 4, DOUT], F32R)
        nc.sync.dma_start(out=wv_s, in_=wv.rearrange("(t p) z -> p t z", p=128))
        wz_s = sing.tile([128, 4, DOUT], F32R)
        nc.sync.dma_start(out=wz_s, in_=wz.rearrange("(t p) z -> p t z", p=128))
        bq_s = sing.tile([128, 4], F32R)
        nc.sync.dma_start(out=bq_s, in_=bq.rearrange("(t p) -> p t", p=128))
        bk_s = sing.tile([128, 4], F32)
        nc.sync.dma_start(out=bk_s, in_=bk.rearrange("(t p) -> p t", p=128))
        bv_s = sing.tile([128, 4], F32)
        nc.sync.dma_start(out=bv_s, in_=bv.rearrange("(t p) -> p t", p=128))
        ident = sing.tile([128, 128], F32)
        make_identity(nc, ident)
        ones_f = sing.tile([1, 128], F32)
        nc.vector.memset(ones_f, 1.0)
        ones_r = sing.tile([1, 128], F32R)
        nc.vector.tensor_copy(ones_r, ones_f)

        for b in range(BPC):
            # ---- batch setup ----
            dT_s = batch_p.tile([128, 4, S], F32R)
            nc.sync.dma_start(out=dT_s,
                              in_=dT[b].rearrange("(t p) k -> p t k", p=128))
            wm_s = batch_p.tile([128, S], F32)
            wm_b = wm[b]
            nc.sync.dma_start(
                out=wm_s,
                in_=bass.AP(tensor=wm_b.tensor, offset=wm_b.offset,
                            ap=[[0, 128]] + wm_b.ap))
            cm_s = batch_p.tile([128, N_SUB], F32)
            nc.sync.dma_start(out=cm_s,
                              in_=cm[b].rearrange("(t p) -> p t", p=128))
            kcm_s = batch_p.tile([128, N_SUB], F32)
            nc.vector.tensor_scalar_mul(kcm_s, cm_s, K_MASK)

            # kT[i,k] = sum_j Wk[j,i] dataT[j,k]  (+bk per-partition)
            kT_s = batch_p.tile([128, 4, S], F32R)
            for it in range(4):
                ps = ps_set.tile([128, S], F32)
                for jt in range(4):
                    nc.tensor.matmul(ps, wk_s[:, jt, it * 128:(it + 1) * 128],
                                     dT_s[:, jt, :],
                                     start=(jt == 0), stop=(jt == 3))
                nc.vector.tensor_scalar_add(kT_s[:, it, :], ps,
                                            bk_s[:, it:it + 1])
            # vsT[z,k] = sum_j Wv[j,z] dataT[j,k]  (+bv per-partition)
            vsT_s = batch_p.tile([128, 4, S], F32R)
            for zt in range(4):
                ps = ps_set.tile([128, S], F32)
                for jt in range(4):
                    nc.tensor.matmul(ps, wv_s[:, jt, zt * 128:(zt + 1) * 128],
                                     dT_s[:, jt, :],
                                     start=(jt == 0), stop=(jt == 3))
                nc.vector.tensor_scalar_add(vsT_s[:, zt, :], ps,
                                            bv_s[:, zt:zt + 1])
            # WqK[d,k] = sum_i Wq[d,i] kT[i,k]
            wqk_s = batch_p.tile([128, 2, S], F32R)
            for dt_ in range(2):
                ps = ps_set.tile([128, S], F32)
                for it in range(4):
                    nc.tensor.matmul(ps, wqt_s[:, it, dt_ * 128:(dt_ + 1) * 128],
                                     kT_s[:, it, :],
                                     start=(it == 0), stop=(it == 3))
                nc.vector.tensor_copy(wqk_s[:, dt_, :], ps)
            # bqk[1,k] = sum_i bq[i] kT[i,k]
            bqk_s = batch_p.tile([1, S], F32R)
            ps = ps_set.tile([128, S], F32)
            for it in range(4):
                nc.tensor.matmul(ps[0:1, :], bq_s[:, it:it + 1], kT_s[:, it, :],
                                 start=(it == 0), stop=(it == 3))
            nc.vector.tensor_copy(bqk_s, ps[0:1, :])
            # VZ[k,z2] = sum_z vsT[z,k] Wz[z,z2]
            vz_s = batch_p.tile([128, 4, DOUT], F32R)
            for kt in range(4):
                ps = ps_set.tile([128, DOUT], F32)
                for zt in range(4):
                    nc.tensor.matmul(ps, vsT_s[:, zt, kt * 128:(kt + 1) * 128],
                                     wz_s[:, zt, :],
                                     start=(zt == 0), stop=(zt == 3))
                nc.vector.tensor_copy(vz_s[:, kt, :], ps)

            # ---- main loop over (q,w) row tiles ----
            for ch in range(N_CH):
                qT_c = chunk_p.tile([128, 2, 128 * SUB_PER_CH], F32R)
                nc.sync.dma_start(
                    out=qT_c,
                    in_=qT[b].rearrange("(t p) c -> p t c", p=128)
                    [:, :, ch * 512:(ch + 1) * 512])
                for sl in range(SUB_PER_CH):
                    st = ch * SUB_PER_CH + sl
                    rows = slice(st * 128, (st + 1) * 128)
                    # scores = bqk (K=1) + sum_d queryT^T WqK
                    psc = ps_sc.tile([128, S], F32)
                    nc.tensor.matmul(psc, ones_r, bqk_s, start=True, stop=False)
                    for dt_ in range(2):
                        nc.tensor.matmul(
                            psc, qT_c[:, dt_, sl * 128:(sl + 1) * 128],
                            wqk_s[:, dt_, :], start=False, stop=(dt_ == 1))
                    # t = (s + K)*cm  (ACT: PSUM->SBUF), then *= wm (DVE)
                    t_s = sub_p.tile([128, S], F32)
                    nc.scalar.activation(
                        out=t_s, in_=psc,
                        func=mybir.ActivationFunctionType.Identity,
                        bias=kcm_s[:, st:st + 1], scale=cm_s[:, st:st + 1])
                    nc.vector.tensor_mul(t_s, t_s, wm_s)
                    mx = stat_p.tile([128, 1], F32)
                    nc.vector.reduce_max(mx, t_s, axis=mybir.AxisListType.X,
                                         negate=True)
                    be = stat_p.tile([128, 1], F32)
                    nc.gpsimd.tensor_scalar_mul(be, mx, INV)
                    # P = exp((t - max)/sqrt(DIN))   (unnormalized)
                    p_s = sub_p.tile([128, S], F32)
                    nc.scalar.activation(
                        out=p_s, in_=t_s,
                        func=mybir.ActivationFunctionType.Exp,
                        bias=be[:, 0:1], scale=INV)
                    nc.sync.dma_start(out=att_o[b, rows, :], in_=p_s)
                    # PT = P^T via TensorE transpose (4x 128x128)
                    ppt = ps_pt.tile([128, S], F32)
                    for kt in range(4):
                        nc.tensor.transpose(
                            ppt[:, kt * 128:(kt + 1) * 128],
                            p_s[:, kt * 128:(kt + 1) * 128], ident)
                    pt_s = sub_p.tile([128, S], F32R)
                    nc.vector.tensor_copy(pt_s, ppt)
                    # zs_un = sum_k PT[k,qw]^T VZ[k,z2]
                    pzs = ps_zs.tile([128, DOUT], F32)
                    for kt in range(4):
                        nc.tensor.matmul(pzs, pt_s[:, kt * 128:(kt + 1) * 128],
                                         vz_s[:, kt, :],
                                         start=(kt == 0), stop=(kt == 3))
                    zs_s = sub_p.tile([128, DOUT], F32)
                    nc.scalar.copy(zs_s, pzs)
                    nc.sync.dma_start(out=zs_o[b, rows, :], in_=zs_s)

    nc.compile()
    return nc


_NC_CACHE = None


def _get_nc():
    global _NC_CACHE
    if _NC_CACHE is None:
        _NC_CACHE = build()
    return _NC_CACHE


def make_in_maps(query, data, content_mask, Wq, bq, Wk, bk, Wv, bv, Wz, bz):
    query = np.ascontiguousarray(query, dtype=np.float32)
    data = np.ascontiguousarray(data, dtype=np.float32)
    cm_full = np.asarray(content_mask).astype(np.float32).reshape(B, QW)
    wm_full = np.asarray(content_mask).any(axis=2).astype(np.float32)
    wqt = np.ascontiguousarray(np.asarray(Wq, dtype=np.float32).T)
    in_maps = []
    for c in range(N_CORES):
        sl = slice(c * BPC, (c + 1) * BPC)
        q_c = query[sl].reshape(BPC, QW, DQ)
        in_maps.append({
            "queryT": np.ascontiguousarray(q_c.transpose(0, 2, 1)),
            "dataT": np.ascontiguousarray(data[sl].transpose(0, 2, 1)),
            "WqT": wqt,
            "Wk": np.ascontiguousarray(Wk, dtype=np.float32),
            "Wv": np.ascontiguousarray(Wv, dtype=np.float32),
            "Wz": np.ascontiguousarray(Wz, dtype=np.float32),
            "bq": np.ascontiguousarray(bq, dtype=np.float32),
            "bk": np.ascontiguousarray(bk, dtype=np.float32),
            "bv": np.ascontiguousarray(bv, dtype=np.float32),
            "cm": np.ascontiguousarray(cm_full[sl]),
            "wm": np.ascontiguousarray(wm_full[sl]),
        })
    return in_maps


def postprocess(results, bz):
    """Gather per-core raw outputs -> full (zs, att) with host normalization."""
    bz = np.asarray(bz, dtype=np.float32)
    att_raw = np.concatenate([r["att"] for r in results], axis=0)  # [B,QW,S]
    zs_raw = np.concatenate([r["zs"] for r in results], axis=0)    # [B,QW,Dout]
    sums = att_raw.sum(axis=-1, dtype=np.float32)                  # [B,QW]
    att_n = att_raw / sums[..., None]
    zs_n = zs_raw / sums[..., None] + bz
    # layouts: att [B,q,k,w] from [B,(q,w),k]; zs [B,S,W,Dout]
    att = np.ascontiguousarray(
        att_n.reshape(B, S, W, S).transpose(0, 1, 3, 2)).astype(np.float32)
    zs = zs_n.reshape(B, S, W, DOUT).astype(np.float32)
    return zs, att


def kernel(query, data, content_mask, Wq, bq, Wk, bk, Wv, bv, Wz, bz):
    nc = _get_nc()
    in_maps = make_in_maps(query, data, content_mask, Wq, bq, Wk, bk,
                           Wv, bv, Wz, bz)
    res = bass_utils.run_bass_kernel_spmd(nc, in_maps,
                                          core_ids=list(range(N_CORES)),
                                          trace=False)
    return postprocess(res.results, bz)
